# revision 70
# baseline (speedup 1.0000x reference)
"""Multi-head attention Trainium2 Bass kernel.

Problem: nn_MultiHeadAttention (B=8, D=256, N=2048, H=4, head_dim=64), fp32.

Sharding: data-parallel over batch — each of the 8 NeuronCores handles one
batch element end to end (no communication needed).

Per-core algorithm:
  - Q/K projections and the score matmuls run in bf16: score noise passes
    through exp() as a tiny multiplicative perturbation of the softmax
    weights (~2e-4), which the value-averaging does not amplify.
  - The V path (V^T projection, PV matmul, output projection) runs in
    float32r (~full PE speed for free-dim >= 256, much better precision
    than bf16) because value-path noise lands on the output directly.
  - Scores are computed transposed, S^T[m, n] = sum_d k[d,m] q[d,n], so no
    operand ever needs a transpose; exp(S^T/8) runs on the scalar engine
    straight out of PSUM (scale fused into the ACTIVATE). Max-subtraction
    is skipped — scores are O(1) here, exp cannot overflow.
  - A ones-column appended to each head's V^T makes the PV matmul emit the
    softmax denominator as an extra PSUM row (row 64); lhsT is padded to 66
    columns because fp32r requires an even stationary free size.
  - fp32r operands must be produced (rounded) by a compute engine, so
    DMA-loaded tensors pass through one DVE copy into bf16/fp32r tiles.
"""

import numpy as np

import concourse.bass as bass
import concourse.bacc as bacc
import concourse.mybir as mybir
import concourse.tile as tile
from concourse.bass_utils import run_bass_kernel_spmd

F32 = mybir.dt.float32
F32R = mybir.dt.float32r
BF16 = mybir.dt.bfloat16
F16 = mybir.dt.float16
EXP = mybir.ActivationFunctionType.Exp

B, D, N, H = 8, 256, 2048, 4
HD = D // H  # 64
P = 128
DC = D // P  # 2 d-chunks
MC = N // P  # 16 m-chunks
NW = 512     # matmul free-dim chunk
WIN = 1024   # exp window (psum scores tile width)
VW = HD + 2  # PV stationary width: 64 v-cols + ones + zero pad (must be even)


def build_nc(debug_taps: bool = False, reps: int = 1, probe: str = '') -> bass.Bass:
    nc = bacc.Bacc()
    assert not debug_taps, "debug taps removed in window-outer version"

    xq_d = nc.declare_dram_parameter("query", [D, N], F32, isOutput=False)
    xk_d = nc.declare_dram_parameter("key", [D, N], F32, isOutput=False)
    xv_d = nc.declare_dram_parameter("value", [D, N], F32, isOutput=False)
    wq_d = nc.declare_dram_parameter("wq", [D, D], F32, isOutput=False)
    wk_d = nc.declare_dram_parameter("wk", [D, D], F32, isOutput=False)
    wv_d = nc.declare_dram_parameter("wv", [D, D], F32, isOutput=False)
    wm_d = nc.declare_dram_parameter("wm", [D, D], F32, isOutput=False)
    bq_d = nc.declare_dram_parameter("bq", [D], F32, isOutput=False)
    bk_d = nc.declare_dram_parameter("bk", [D], F32, isOutput=False)
    bv_d = nc.declare_dram_parameter("bv", [D], F32, isOutput=False)
    bm_d = nc.declare_dram_parameter("bm", [D], F32, isOutput=False)
    out_d = nc.declare_dram_parameter("out", [D, N], F32, isOutput=True)

    with tile.TileContext(nc) as tc:
        for _rep in range(reps):
            with (
                tc.tile_pool(name="persist", bufs=1) as pp,
                tc.tile_pool(name="stage", bufs=2) as sp,
            ):
                isp = tc.alloc_tile_pool(name="instage", bufs=1)
                # ---- load + round inputs ----------------------------------------
                # fp32r/bf16 matmul operands must be rounded by a compute engine,
                # so every DMA-loaded tensor passes through one DVE copy. Each
                # input gets its own stage tile so the input DMAs carry no sync
                # waits (the HWDGE DMA pseudo-instruction has very few wait slots).
                def load_round(dram_ap, shape, dtype, name, split=1, dma=None):
                    st = isp.tile(shape, F32, tag=f"st_{name}", name=f"st_{name}")
                    t = pp.tile(shape, dtype, name=name)
                    # split along dim 1 so consumers of the first chunk start
                    # before the whole tensor is staged + rounded
                    step = shape[1] // split
                    dmas = dma if isinstance(dma, list) else [dma or nc.sync]
                    for i, s0 in enumerate(range(0, shape[1], step)):
                        sl = slice(s0, s0 + step)
                        dmas[i % len(dmas)].dma_start(st[:, sl], dram_ap[:, sl])
                        nc.vector.tensor_copy(t[:, sl], st[:, sl])
                    return t

                wq_b = load_round(
                    wq_d.rearrange("(dc p) o -> p dc o", p=P), [P, DC, D], F16, "wq_b"
                )
                xq_b = load_round(
                    xq_d.rearrange("(dc p) n -> p dc n", p=P), [P, DC, N], F16, "xq_b", split=DC
                )
                wk_b = load_round(
                    wk_d.rearrange("(dc p) o -> p dc o", p=P), [P, DC, D], F16, "wk_b",
                    dma=nc.scalar,
                )
                xk_b = load_round(
                    xk_d.rearrange("(dc p) n -> p dc n", p=P), [P, DC, N], F16, "xk_b",
                    split=DC, dma=nc.scalar,
                )
                # V-path loads ride the otherwise-idle ACT-HWDGE queue so the
                # q/k stream on the sync queue gates the attention start alone
                wv_r = load_round(
                    wv_d.rearrange("(dc p) o -> p dc o", p=P), [P, DC, D], F32R, "wv_r",
                    dma=nc.scalar,
                )
                xv_r = load_round(
                    xv_d.rearrange("(dc p) n -> p dc n", p=P), [P, DC, N], F32R, "xv_r",
                    split=DC, dma=[nc.sync, nc.scalar],
                )

                bv_bc = pp.tile([P, D], F32)
                nc.sync.dma_start(
                    bv_bc[:], bv_d[:].rearrange("(a o) -> a o", a=1).to_broadcast((P, D))
                )
                bq_sb = pp.tile([P, DC], F32)
                nc.sync.dma_start(bq_sb[:], bq_d.rearrange("(c p) -> p c", p=P))
                bk_sb = pp.tile([P, DC], F32)
                nc.sync.dma_start(bk_sb[:], bk_d.rearrange("(c p) -> p c", p=P))
                bm_sb = pp.tile([P, DC], F32)

                # warm the exp activation-table while input DMAs stream: the
                # ~2.7us ACT_TABLE_LOAD fires before the first Exp in ACT
                # program order, so a dummy exp here pulls it off the
                # attention critical path (ACT is otherwise idle at start).
                warm = pp.tile([1, 2], F32)
                nc.vector.memset(warm[:], 0.0)
                nc.scalar.activation(warm[:], warm[:], EXP, scale=0.125)

                # ---- persistent compute tiles -----------------------------------
                q_sb = pp.tile([P, DC, N], F16)
                k_sb = pp.tile([P, DC, N], F16)
                vT_sb = pp.tile([P, MC, H, VW], F32R)
                # memset can't write float32r — round a small f32 [1, 0] pair in
                ones2 = pp.tile([P, 2], F32)
                nc.vector.memset(ones2[:, 0:1], 1.0)
                nc.vector.memset(ones2[:, 1:2], 0.0)
                nc.vector.tensor_copy(
                    vT_sb[:, :, :, HD : HD + 2],
                    ones2.unsqueeze(1).unsqueeze(1).to_broadcast((P, MC, H, 2)),
                )
                xst_sb = pp.tile([HD, H, N], F32R)  # normalized per-head attn out

                isp.release()  # staging range reused by the attention pools below

                # ---- projections -------------------------------------------------
                # q/k chunk 0 first so head-0 attention can start early, then
                # v^T (PV consumes it m-chunk by m-chunk), then q/k chunk 1.
                with tc.tile_pool(name="psum_proj", bufs=2, space="PSUM") as pjp:

                    def emit_qk(w_sb, x_sb, b_sb, dst, oc):
                        for nw in range(N // NW):
                            ps_p = pjp.tile([P, NW], F32, tag="pqk", name="ps_p")
                            for dc in range(DC):
                                nc.tensor.matmul(
                                    ps_p[:],
                                    w_sb[:, dc, oc * P : (oc + 1) * P],
                                    x_sb[:, dc, nw * NW : (nw + 1) * NW],
                                    start=(dc == 0),
                                    stop=(dc == DC - 1),
                                )
                            nc.vector.tensor_add(
                                out=dst[:, oc, nw * NW : (nw + 1) * NW],
                                in0=ps_p[:],
                                in1=b_sb[:, oc : oc + 1].to_broadcast((P, NW)),
                            )

                    emit_qk(wq_b, xq_b, bq_sb, q_sb, 0)
                    emit_qk(wk_b, xk_b, bk_sb, k_sb, 0)
                    emit_qk(wq_b, xq_b, bq_sb, q_sb, 1)
                    emit_qk(wk_b, xk_b, bk_sb, k_sb, 1)

                    # v^T AFTER both q/k chunks: on HW (~75 GB/s/core DMA) the
                    # attention start is gated by every projection emitted
                    # before it in PE program order; q/k alone are 4.5MB vs
                    # 6.75MB with xv, so first exp fires ~20us earlier and the
                    # v chunks stream in behind while attention runs
                    for mc in range(MC):
                        ps_v = pjp.tile([P, D], F32, tag="pv")
                        for dc in range(DC):
                            nc.tensor.matmul(
                                ps_v[:],
                                xv_r[:, dc, mc * P : (mc + 1) * P],
                                wv_r[:, dc, :],
                                start=(dc == 0),
                                stop=(dc == DC - 1),
                            )
                        nc.vector.tensor_add(
                            out=vT_sb[:, mc, :, 0:HD],
                            in0=ps_v[:].rearrange("p (h e) -> p h e", e=HD),
                            in1=bv_bc[:].rearrange("p (h e) -> p h e", e=HD),
                        )


                # ---- attention ---------------------------------------------------
                with (
                    tc.tile_pool(name="psum_att", bufs=1, space="PSUM") as pa,
                    tc.tile_pool(name="exp_pool", bufs=6) as ep,
                    tc.tile_pool(name="rbc_pool", bufs=3) as rp,
                    tc.tile_pool(name="dram_scr", bufs=4, space="DRAM") as dsp,
                ):
                    # Head-pair processing: the two heads of each q/k chunk
                    # live at partition bases 0 and 64, so their score matmuls
                    # target different PE row groups and overlap in the array
                    # (weight loads included). Window-outer keeps two (66, WIN)
                    # x accumulators + double-buffered score tiles in 8 banks.
                    for hc in range(DC):
                        for w in range(N // WIN):
                            x_ps = [
                                pa.tile([VW, WIN], F32, tag=f"x{i}", bufs=1, name="x_ps")
                                for i in range(2)
                            ]

                            def emit_pv(mc, e_pair):
                                for i in range(2):
                                    for j in range(WIN // NW):
                                        nc.tensor.matmul(
                                            x_ps[i][:, j * NW : (j + 1) * NW],
                                            vT_sb[:, mc, hc * 2 + i, :],
                                            e_pair[i][:, j * NW : (j + 1) * NW],
                                            start=(mc == 0),
                                            stop=(mc == MC - 1),
                                        )

                            prev = None
                            for mc in range(MC):
                                e_pair = []
                                for i in range(2):
                                    hb = i * HD
                                    s_ps = pa.tile(
                                        [P, WIN], F32, tag="s", bufs=2, name="s_ps"
                                    )
                                    for j in range(WIN // NW):
                                        n0 = w * WIN + j * NW
                                        nc.tensor.matmul(
                                            s_ps[:, j * NW : (j + 1) * NW],
                                            k_sb[hb : hb + HD, hc, mc * P : (mc + 1) * P],
                                            q_sb[hb : hb + HD, hc, n0 : n0 + NW],
                                            start=True,
                                            stop=True,
                                        )
                                    e_sb = ep.tile([P, WIN], F32R, tag="e", name="e_sb")
                                    nc.scalar.activation(
                                        e_sb[:], s_ps[:], EXP, scale=0.125
                                    )
                                    e_pair.append(e_sb)
                                if prev is not None:
                                    emit_pv(*prev)
                                prev = (mc, e_pair)
                            emit_pv(*prev)

                            # epilogue per head: one (65, WIN) DVE copy moves
                            # x_unnorm + denominator out of PSUM; reciprocal is
                            # broadcast to partitions 0..63 via a DRAM bounce.
                            n0 = w * WIN
                            for i in range(2):
                                h = hc * 2 + i
                                xu = rp.tile(
                                    [HD + 1, WIN], F32, tag="xu", bufs=3, name="xu"
                                )
                                nc.vector.tensor_copy(xu[:], x_ps[i][0 : HD + 1, :])
                                rden_dr = dsp.tile(
                                    [1, WIN], F32, tag="dden", name="rden_dr"
                                )
                                nc.gpsimd.dma_start(rden_dr[:], xu[HD : HD + 1, :])
                                rden_bc = rp.tile(
                                    [HD, WIN], F32, tag="rbc", name="rden_bc"
                                )
                                nc.gpsimd.dma_start(
                                    rden_bc[:], rden_dr[:].to_broadcast((HD, WIN))
                                )
                                nc.vector.reciprocal_approx_fast(
                                    out=rden_bc[:], in_=rden_bc[:]
                                )
                                nc.vector.tensor_mul(
                                    out=xst_sb[:, h, n0 : n0 + WIN],
                                    in0=xu[0:HD, :],
                                    in1=rden_bc[:],
                                )

                # ---- output projection ------------------------------------------
                # wm/bm are only needed here — loading them late keeps their
                # 0.26MB out of the head-critical input DMA stream
                st_wm = sp.tile([HD, H, D], F32, tag="stwm", name="st_wm")
                nc.sync.dma_start(st_wm[:], wm_d.rearrange("(h p) o -> p h o", p=HD))
                wm_r = pp.tile([HD, H, D], F32R, name="wm_r")
                nc.vector.tensor_copy(wm_r[:], st_wm[:])
                nc.sync.dma_start(bm_sb[:], bm_d.rearrange("(c p) -> p c", p=P))
                with tc.tile_pool(name="psum_out", bufs=4, space="PSUM") as po:
                    for oc in range(DC):
                        # 4 concurrent accumulators so each wm slice is loaded
                        # once and streams all four n-chunks (h loop outer)
                        ps_os = [
                            po.tile([P, NW], F32, tag="po", name="ps_o")
                            for _ in range(N // NW)
                        ]
                        for h in range(H):
                            for nw in range(N // NW):
                                nc.tensor.matmul(
                                    ps_os[nw][:],
                                    wm_r[:, h, oc * P : (oc + 1) * P],
                                    xst_sb[:, h, nw * NW : (nw + 1) * NW],
                                    start=(h == 0),
                                    stop=(h == H - 1),
                                )
                        for nw in range(N // NW):
                            o_sb = sp.tile([P, NW], F32, tag="ostage", name="o_sb")
                            nc.vector.tensor_add(
                                out=o_sb[:],
                                in0=ps_os[nw][:],
                                in1=bm_sb[:, oc : oc + 1].to_broadcast((P, NW)),
                            )
                            nc.sync.dma_start(
                                out_d.rearrange("(c p) n -> p c n", p=P)[
                                    :, oc, nw * NW : (nw + 1) * NW
                                ],
                                o_sb[:],
                            )

    nc.finalize()
    return nc


_NC_CACHE = None


def _get_nc():
    global _NC_CACHE
    if _NC_CACHE is None:
        _NC_CACHE = build_nc()
    return _NC_CACHE


# column j of the permuted Wq/Wk maps to original output channel o = hd*H + h
# with j = (h // 2) * 128 + (h % 2) * 64 + hd  (head-contiguous, chunk-split)
_QK_PERM = np.empty(D, np.int64)
for _j in range(D):
    _c, _rr = divmod(_j, P)
    _h2, _hd = divmod(_rr, HD)
    _QK_PERM[_j] = _hd * H + (_c * 2 + _h2)
# column j of the permuted Wv maps to o = hd*H + h with j = h*64 + hd
_V_PERM = np.empty(D, np.int64)
for _j in range(D):
    _h, _hd = divmod(_j, HD)
    _V_PERM[_j] = _hd * H + _h


def make_in_maps(**inputs: np.ndarray) -> list:
    query = np.ascontiguousarray(np.asarray(inputs["query"], np.float32))
    key = np.ascontiguousarray(np.asarray(inputs["key"], np.float32))
    value = np.ascontiguousarray(np.asarray(inputs["value"], np.float32))
    wq = np.ascontiguousarray(np.asarray(inputs["Wq"], np.float32)[:, _QK_PERM])
    wk = np.ascontiguousarray(np.asarray(inputs["Wk"], np.float32)[:, _QK_PERM])
    wv = np.ascontiguousarray(np.asarray(inputs["Wv"], np.float32)[:, _V_PERM])
    wm = np.ascontiguousarray(np.asarray(inputs["Wm"], np.float32)[_V_PERM, :])
    bq = np.ascontiguousarray(np.asarray(inputs["bq"], np.float32)[_QK_PERM])
    bk = np.ascontiguousarray(np.asarray(inputs["bk"], np.float32)[_QK_PERM])
    bv = np.ascontiguousarray(np.asarray(inputs["bv"], np.float32)[_V_PERM])
    bm = np.ascontiguousarray(np.asarray(inputs["bm"], np.float32))

    return [
        {
            "query": query[b],
            "key": key[b],
            "value": value[b],
            "wq": wq,
            "wk": wk,
            "wv": wv,
            "wm": wm,
            "bq": bq,
            "bk": bk,
            "bv": bv,
            "bm": bm,
        }
        for b in range(B)
    ]


def kernel(**inputs: np.ndarray) -> np.ndarray:
    nc = _get_nc()
    in_maps = make_in_maps(**inputs)
    res = run_bass_kernel_spmd(nc, in_maps, core_ids=list(range(B)))
    global _LAST_RESULT
    _LAST_RESULT = res
    return np.stack([r["out"] for r in res.results], axis=0)


_LAST_RESULT = None

